# revision 35
# baseline (speedup 1.0000x reference)
"""Sharded GQA attention (causal + packed-segment mask) for 8 Trainium2 NeuronCores.

Strategy (v4)
-------------
* Core c handles batch b = c//4 and KV heads {2*(c%4), 2*(c%4)+1} (8 query
  heads per core); the sequence dim stays unsharded.
* decoder_segment_ids are sorted, so attention is block-diagonal over
  contiguous segments; the device kernel does causal-only attention per
  segment over 128-wide chunks.  The two batches' run structures are
  unioned so all 8 cores execute one SPMD program.
* dtypes: QK matmuls run float16 (or float32r via qdt config); P (post-exp)
  and V are bf16 so the 130-col PV matmuls stream 1 col/cycle; output is
  bf16 (host upcasts).  Expected end-to-end rel err ~6.5e-3.
* No mask matmuls: ghost rows/columns self-neutralise (zero K rows give
  S=0 -> P=1, but the matching V rows and ones-column are zero), so only
  the causal mask inside each diagonal 128x128 block is needed.  It is a
  single shared bf16 0/1 tile applied post-exp with one tensor_tensor
  multiply per diagonal chunk, split between DVE and GPSIMD.
* Q is packed host-side to only-real columns; QK, exp and normalize are
  trimmed to real columns.
* exp runs once per slab (t-block) over a [128, (j+1), 4*nr] PSUM slab
  (chunk-per-bank); softmax denominators fall out of the PV matmuls via a
  bf16 ones-column appended to V; the normalize is one 4D broadcast
  tensor_mul per slab on DVE.
* DMA-issue overhead (~1.2us per DMA of SEQ+DGE time) dominates at this
  scale, so all per-(i,kv) inputs (K^T, packed Q^T, V) ride in ONE
  uint16-packed DMA with bitcast views, and each (i,kv)'s four output
  slabs leave in one DMA from a staging tile: 13 DMAs per iteration.
"""

import math

import numpy as np
import ml_dtypes

B, T, NQ, NKV, D = 2, 1024, 32, 8, 128
G = NQ // NKV
NCORES = 8
KV_PER_CORE = NKV // (NCORES // B)
CHUNK = 128
BF16 = ml_dtypes.bfloat16

QDT = "f16"           # "f32r" or "f16" for the QK matmul dtype
MASK_MODE = "pe"      # "pe": additive NEG mask matmul fused into the QK
                      # accumulation; "dve": 0/1 multiply post-exp
MASK_GP_FRAC = 0.72   # dve mode: fraction of mask multiplies on GPSIMD
NEG = -1.0e9
CG = 1                # chunks per PSUM slab tile (banks each)
SLAB_BUFS = 4         # psum_s pool buffers
OT_BUFS = 2           # psum_o pool buffers
DMA_SPLIT = True      # split each (i,kv) input DMA into (k+q | v) pieces so
                      # the first QK starts before V lands
OUT_SPLIT = True      # per-slab output DMAs (smaller tail) instead of
                      # one DMA per (i,kv)

_PROGRAM_CACHE = {}


# --------------------------------------------------------------------------
# host-side structure
# --------------------------------------------------------------------------

def _runs(seg_row):
    d = np.flatnonzero(np.diff(seg_row) != 0)
    starts = np.concatenate(([0], d + 1))
    ends = np.concatenate((d + 1, [len(seg_row)]))
    return [(int(s), int(e - s)) for s, e in zip(starts, ends)]


def _structure(ids):
    runs = [_runs(np.asarray(ids[b])) for b in range(B)]
    n_seg = max(len(r) for r in runs)
    L = [max((r[i][1] for r in runs if len(r) > i), default=0) for i in range(n_seg)]
    K = [math.ceil(l / CHUNK) for l in L]
    segs = [i for i in range(n_seg) if K[i] > 0]
    slabs = [(i, kv_i, j) for i in segs for kv_i in range(KV_PER_CORE)
             for j in range(K[i])]
    chunks = [(i, kv_i, c) for i in segs for kv_i in range(KV_PER_CORE)
              for c in range(K[i])]
    # real (non-ghost) q columns of slab (i, kv_i, j), from the union lengths
    nr = {(i, kv_i, j): min(CHUNK, L[i] - j * CHUNK)
          for (i, kv_i, j) in slabs}
    qbase = {}
    acc = 0
    for s in slabs:
        qbase[s] = acc
        acc += G * nr[s]
    return runs, L, K, segs, slabs, chunks, nr, qbase, acc


def _ikv_layout(K, slabs, chunks, nr, qbase):
    """Per-(i,kv) packed-input column layout (units: 2-byte elements)."""
    chunk_idx = {c: i for i, c in enumerate(chunks)}
    ikvs = sorted({(i, kv_i) for (i, kv_i, _) in slabs})
    lay = {}
    base = 0
    for (i, kv_i) in ikvs:
        kk = K[i]
        qlen = sum(G * nr[(i, kv_i, j)] for j in range(kk))
        kcols = kk * CHUNK
        vcols = kk * 130
        lay[(i, kv_i)] = dict(base=base, kcols=kcols, qlen=qlen, vcols=vcols,
                              ci0=chunk_idx[(i, kv_i, 0)], kk=kk)
        base += kcols + qlen + vcols
    return ikvs, lay, base


def _prepare_core(core, q, k, v, runs, L, K, segs, slabs, chunks, nr, qbase,
                  qcols, qdt=QDT):
    b = core // (NCORES // B)
    kv_heads = [KV_PER_CORE * (core % (NCORES // B)) + x for x in range(KV_PER_CORE)]
    rb = runs[b]
    np_qdt = np.float32 if qdt == "f32r" else np.float16

    def seg_info(i):
        if i < len(rb):
            return rb[i]
        return (0, 0)

    qT = np.zeros((D, qcols), np_qdt)
    for s in slabs:
        i, kv_i, j = s
        a, lb = seg_info(i)
        t0 = j * CHUNK
        n_real = min(nr[s], max(lb - t0, 0))
        if n_real > 0:
            base = qbase[s]
            for g in range(G):
                h = G * kv_heads[kv_i] + g
                blk = q[b, a + t0:a + t0 + n_real, h, :]  # [n_real, D]
                qT[:, base + g * nr[s]: base + g * nr[s] + n_real] = blk.T

    kT = np.zeros((D, len(chunks) * CHUNK), np_qdt)
    vO = np.zeros((CHUNK, len(chunks) * 130), BF16)
    for ci, (i, kv_i, c) in enumerate(chunks):
        a, lb = seg_info(i)
        s0 = c * CHUNK
        n_real = min(CHUNK, lb - s0)
        if n_real > 0:
            kvh = kv_heads[kv_i]
            kT[:, ci * CHUNK: ci * CHUNK + n_real] = \
                k[b, a + s0:a + s0 + n_real, kvh, :].T.astype(np_qdt)
            vO[:n_real, ci * 130: ci * 130 + D] = \
                v[b, a + s0:a + s0 + n_real, kvh, :].astype(BF16)
            vO[:n_real, ci * 130 + D] = BF16(1.0)

    sr = np.arange(CHUNK)
    if MASK_MODE == "pe":
        keep = np.where(sr[:, None] > sr[None, :], np.float32(NEG),
                        np.float32(0.0))  # additive: NEG where t < s
    else:
        keep = (sr[:, None] <= sr[None, :]).astype(np.float32)  # 0/1 keep
    mask = np.concatenate([keep] * G, axis=1).astype(BF16)  # [s, g*128 + t]

    return {"qT": qT, "kT": kT, "vO": vO, "mask": mask,
            "ident": np.eye(CHUNK, dtype=BF16)}


def _pack_core(ci, K, slabs, chunks, nr, qbase, qdt=QDT):
    """Build the device in_map from the logical per-core arrays."""
    ikvs, lay, total = _ikv_layout(K, slabs, chunks, nr, qbase)
    if qdt == "f16":
        inb = np.zeros((CHUNK, total), np.uint16)
        for ikv in ikvs:
            l = lay[ikv]
            b0 = l["base"]
            ci0, kk = l["ci0"], l["kk"]
            s0 = (ikv[0], ikv[1], 0)
            inb[:, b0:b0 + l["kcols"]] = \
                ci["kT"][:, ci0 * CHUNK:(ci0 + kk) * CHUNK].view(np.uint16)
            b1 = b0 + l["kcols"]
            inb[:, b1:b1 + l["qlen"]] = \
                ci["qT"][:, qbase[s0]: qbase[s0] + l["qlen"]].view(np.uint16)
            b2 = b1 + l["qlen"]
            inb[:, b2:b2 + l["vcols"]] = \
                ci["vO"][:, ci0 * 130:(ci0 + kk) * 130].view(np.uint16)
        return {"inb": inb, "mask": ci["mask"], "ident": ci["ident"]}
    return {"kT": ci["kT"], "qT": ci["qT"], "vO": ci["vO"],
            "mask": ci["mask"], "ident": ci["ident"]}


def _assemble(outs, runs, slabs, nr):
    full = np.zeros((B, T, NQ, D), np.float32)
    for core in range(NCORES):
        b = core // (NCORES // B)
        kv_heads = [KV_PER_CORE * (core % (NCORES // B)) + x
                    for x in range(KV_PER_CORE)]
        res = outs[core]  # [NSLAB, 128, 512] bf16
        rb = runs[b]
        for si, (i, kv_i, j) in enumerate(slabs):
            if i >= len(rb):
                continue
            a, lb = rb[i]
            t0 = j * CHUNK
            n_real = min(CHUNK, lb - t0)
            if n_real <= 0:
                continue
            for g in range(G):
                h = G * kv_heads[kv_i] + g
                full[b, a + t0:a + t0 + n_real, h, :] = \
                    res[si, :n_real, g * CHUNK:g * CHUNK + D].astype(np.float32)
    return full


# --------------------------------------------------------------------------
# numpy emulation of the device schedule (debug/validation only)
# --------------------------------------------------------------------------

def _numpy_schedule(ins, L, K, segs, slabs, chunks, nr, qbase):
    chunk_idx = {c: i for i, c in enumerate(chunks)}
    qT = ins["qT"].astype(np.float32)
    kT = ins["kT"].astype(np.float32)
    vO = ins["vO"].astype(np.float32)
    mask = ins["mask"].astype(np.float32)
    out = np.zeros((len(slabs), CHUNK, G * CHUNK), BF16)
    for si, (i, kv_i, j) in enumerate(slabs):
        n = nr[(i, kv_i, j)]
        qt = qT[:, qbase[(i, kv_i, j)]: qbase[(i, kv_i, j)] + G * n]  # [d, 4n]
        ot = np.zeros((CHUNK, G, 130), np.float32)
        for c in range(j + 1):
            ci = chunk_idx[(i, kv_i, c)]
            lhsT = kT[:, ci * CHUNK:(ci + 1) * CHUNK]          # [d, s]
            S = lhsT.T @ qt                                    # [s, 4n]
            m = np.concatenate([mask[:, :n]] * G, axis=1)      # [s, 4n]
            if MASK_MODE == "pe":
                if c == j:
                    S = S + m
                P = np.exp(S)
            else:
                P = np.exp(S)
                if c == j:
                    P = P * m
            P = P.astype(BF16).astype(np.float32)
            vo = vO[:, ci * 130:ci * 130 + 130]                # [s, 130]
            for g in range(G):
                ot[:n, g, :] += P[:, g * n:(g + 1) * n].T @ vo
        den = ot[:, :, D]
        with np.errstate(divide="ignore", invalid="ignore"):
            recip = 1.0 / den
            norm = ot[:, :, :D] * recip[:, :, None]
        out[si, :, :] = norm.reshape(CHUNK, G * D).astype(BF16)
    return out


# --------------------------------------------------------------------------
# bass program
# --------------------------------------------------------------------------

def _build_program(L, K, segs, slabs, chunks, nr, qbase, qcols, qdt=QDT,
                   loop_n=0, unroll=1):
    import contextlib

    import concourse.bacc as bacc
    import concourse.bass as bass
    import concourse.tile as tile
    from concourse import mybir

    f32 = mybir.dt.float32
    bf16 = mybir.dt.bfloat16
    u16 = mybir.dt.uint16
    f16pack = qdt == "f16"
    mm_dt = mybir.dt.float32r if qdt == "f32r" else mybir.dt.float16
    maxK = max(K[i] for i in segs)
    nslab = len(slabs)
    nchunk = len(chunks)
    ikvs, lay, packed_cols = _ikv_layout(K, slabs, chunks, nr, qbase)

    nc = bacc.Bacc()
    if f16pack:
        inb_d = nc.dram_tensor("inb", [CHUNK, packed_cols], u16,
                               kind="ExternalInput")
    else:
        qT_d = nc.dram_tensor("qT", [D, qcols], mm_dt, kind="ExternalInput")
        kT_d = nc.dram_tensor("kT", [D, nchunk * CHUNK], mm_dt,
                              kind="ExternalInput")
        vO_d = nc.dram_tensor("vO", [CHUNK, nchunk * 130], bf16,
                              kind="ExternalInput")
    mask_d = nc.dram_tensor("mask", [CHUNK, G * CHUNK], bf16,
                            kind="ExternalInput")
    ident_d = nc.dram_tensor("ident", [CHUNK, CHUNK], bf16,
                             kind="ExternalInput")
    out_d = nc.dram_tensor("out", [nslab, CHUNK, G * CHUNK], bf16,
                           kind="ExternalOutput")
    slab_idx = {s: i for i, s in enumerate(slabs)}

    with tile.TileContext(nc) as tc:
      with tc.tile_pool(name="pin", bufs=1) as pin, \
           tc.tile_pool(name="pp", bufs=3) as pp, \
           tc.tile_pool(name="po", bufs=2) as po, \
           tc.tile_pool(name="psum_s", bufs=SLAB_BUFS, space="PSUM") as psum_s, \
           tc.tile_pool(name="psum_o", bufs=OT_BUFS, space="PSUM") as psum_o:
        # loop-invariant: causal mask + identity, loaded once; issued from
        # the ACT queue so the SP queue's first input DMA is not delayed
        mask_t = pin.tile([CHUNK, G * CHUNK], bf16, tag="mask")
        nc.scalar.dma_start(out=mask_t[:], in_=mask_d[:])
        ident_t = pin.tile([CHUNK, CHUNK], bf16, tag="ident")
        nc.scalar.dma_start(out=ident_t[:], in_=ident_d[:])
        with (tc.For_i(0, loop_n, 1) if loop_n else contextlib.nullcontext()):
          for _it in range(max(1, unroll)):
            # one packed input DMA per (i,kv), in consumption order, so the
            # For_i loop's n+1 DMAs overlap iteration n's compute
            kT_t, qT_t, vO_t = {}, {}, {}
            for ikv in ikvs:
                l = lay[ikv]
                kk = l["kk"]
                if f16pack:
                    icols = l["kcols"] + l["qlen"] + l["vcols"]
                    kqc = l["kcols"] + l["qlen"]
                    it = pin.tile([CHUNK, icols], u16,
                                  tag=f"in_{ikv[0]}_{ikv[1]}")
                    if DMA_SPLIT:
                        if ikv == ikvs[0]:
                            # first stream extra-fine: K, then the first
                            # slab's Q, so the first QK starts ASAP
                            kc = l["kcols"]
                            q0 = G * nr[(ikv[0], ikv[1], 0)]
                            nc.sync.dma_start(
                                out=it[:, 0:kc],
                                in_=inb_d[:, l["base"]: l["base"] + kc])
                            nc.sync.dma_start(
                                out=it[:, kc:kc + q0],
                                in_=inb_d[:, l["base"] + kc:
                                           l["base"] + kc + q0])
                            nc.sync.dma_start(
                                out=it[:, kc + q0:kqc],
                                in_=inb_d[:, l["base"] + kc + q0:
                                           l["base"] + kqc])
                        else:
                            nc.sync.dma_start(
                                out=it[:, 0:kqc],
                                in_=inb_d[:, l["base"]: l["base"] + kqc])
                        nc.sync.dma_start(
                            out=it[:, kqc:icols],
                            in_=inb_d[:, l["base"] + kqc: l["base"] + icols])
                    else:
                        nc.sync.dma_start(
                            out=it[:],
                            in_=inb_d[:, l["base"]: l["base"] + icols])
                    kT_t[ikv] = it[:, 0:l["kcols"]].bitcast(mm_dt)
                    qT_t[ikv] = it[:, l["kcols"]: l["kcols"] + l["qlen"]] \
                        .bitcast(mm_dt)
                    vO_t[ikv] = it[:, l["kcols"] + l["qlen"]: icols] \
                        .bitcast(bf16)
                else:
                    ci0 = l["ci0"]
                    s0 = (ikv[0], ikv[1], 0)
                    kt = pin.tile([D, kk * CHUNK], mm_dt,
                                  tag=f"kT_{ikv[0]}_{ikv[1]}")
                    nc.sync.dma_start(
                        out=kt[:], in_=kT_d[:, ci0 * CHUNK:(ci0 + kk) * CHUNK])
                    kT_t[ikv] = kt[:]
                    qt = pin.tile([D, l["qlen"]], mm_dt,
                                  tag=f"qT_{ikv[0]}_{ikv[1]}")
                    nc.sync.dma_start(
                        out=qt[:], in_=qT_d[:, qbase[s0]: qbase[s0] + l["qlen"]])
                    qT_t[ikv] = qt[:]
                    vt = pin.tile([CHUNK, kk * 130], bf16,
                                  tag=f"vO_{ikv[0]}_{ikv[1]}")
                    nc.sync.dma_start(
                        out=vt[:], in_=vO_d[:, ci0 * 130:(ci0 + kk) * 130])
                    vO_t[ikv] = vt[:]

            # ---- software-pipelined wavefront over the (i,kv) streams ----
            # Streams are independent; stagger them by one j-step and emit
            # stage1 (QK+exp+mask) of step t before stage2 (PV+normalize)
            # of step t-1, so every engine's in-order queue always holds
            # dependency-resolved work.
            mask_state = {"idx": 0}
            ost_t = {}

            def stage1(i, kv_i, j):
                kt = kT_t[(i, kv_i)]
                n = nr[(i, kv_i, j)]
                fcols = G * n
                qoff = qbase[(i, kv_i, j)] - qbase[(i, kv_i, 0)]
                qt = qT_t[(i, kv_i)][:, qoff: qoff + fcols]
                m_ap = bass.AP(tensor=mask_t.tensor, offset=mask_t.offset,
                               ap=[mask_t.ap[0], [0, G], [1, n]])
                pts = []  # (pt_tile, c0, glen)
                for c0 in range(0, j + 1, CG):
                    glen = min(CG, j + 1 - c0)
                    slab = psum_s.tile([CHUNK, CG, G * CHUNK], f32,
                                       tag="slab")
                    for gi in range(glen):
                        c = c0 + gi
                        masked = MASK_MODE == "pe" and c == j
                        nc.tensor.matmul(
                            slab[:, gi, 0:fcols],
                            kt[:, c * CHUNK:(c + 1) * CHUNK], qt,
                            start=True, stop=not masked)
                        if masked:
                            # accumulate the additive NEG causal mask into
                            # the diagonal chunk's S (same PSUM group, no
                            # cross-engine hop; exp then emits exact zeros)
                            sl3 = slab[:, gi, 0:fcols] \
                                .rearrange("p (g t) -> p g t", g=G)
                            nc.tensor.matmul(sl3, ident_t[:], m_ap,
                                             start=False, stop=True)
                    pt = pp.tile([CHUNK, CG * G * CHUNK], bf16, tag="pt",
                                 bufs=8)
                    nc.scalar.activation(
                        out=pt[:, 0:glen * fcols]
                            .rearrange("p (k c) -> p k c", k=glen),
                        in_=slab[:, 0:glen, 0:fcols],
                        func=mybir.ActivationFunctionType.Exp)
                    pts.append((pt, c0, glen))

                if MASK_MODE == "dve":
                    # causal mask on the diagonal chunk (post-exp)
                    pt_j, c0_j, _ = pts[-1]
                    diag_off = (j - c0_j) * fcols
                    diag = pt_j[:, diag_off: diag_off + fcols] \
                        .rearrange("p (g t) -> p g t", g=G)
                    mi = mask_state["idx"]
                    gp_due = int(round((mi + 1) * MASK_GP_FRAC)) \
                        - int(round(mi * MASK_GP_FRAC))
                    eng = nc.gpsimd if gp_due else nc.vector
                    eng.tensor_mul(out=diag, in0=diag, in1=m_ap)
                    mask_state["idx"] = mi + 1
                return pts

            def stage2(i, kv_i, j, pts):
                kk = K[i]
                n = nr[(i, kv_i, j)]
                fcols = G * n
                vt = vO_t[(i, kv_i)]
                if j == 0:
                    ost = po.tile([CHUNK, kk * G * CHUNK], bf16,
                                  tag=f"ost_{i}_{kv_i}", bufs=2,
                                  name=f"ost_{i}_{kv_i}")
                    ost_t[(i, kv_i)] = ost
                ost = ost_t[(i, kv_i)]

                ot = psum_o.tile([CHUNK, 2, 512], f32, tag="ot")
                for c in range(j + 1):
                    pt, c0, _ = pts[c // CG]
                    poff = (c - c0) * fcols
                    vsl = vt[:, c * 130:c * 130 + 130]
                    for g in range(G):
                        nc.tensor.matmul(
                            ot[0:n, g // 2,
                               (g % 2) * 132:(g % 2) * 132 + 130],
                            pt[:, poff + g * n: poff + (g + 1) * n], vsl,
                            start=(c == 0 and g % 2 == 0),
                            stop=(c == j and g % 2 == 1))

                # normalize (DVE): recip + broadcast multiply into staging
                recip = po.tile([CHUNK, G], f32, tag="recip", bufs=4)
                den_ap = bass.AP(tensor=ot.tensor, offset=ot.offset + D,
                                 ap=[ot.ap[0], [512, 2], [132, 2]])
                r4 = bass.AP(tensor=recip.tensor, offset=recip.offset,
                             ap=[recip.ap[0], [2, 2], [1, 2]])
                nc.vector.reciprocal(out=r4, in_=den_ap)
                obase = j * G * CHUNK
                out_ap = bass.AP(tensor=ost.tensor,
                                 offset=ost.offset + obase,
                                 ap=[ost.ap[0], [2 * D, 2], [D, 2], [1, D]])
                num_ap = bass.AP(tensor=ot.tensor, offset=ot.offset,
                                 ap=[ot.ap[0], [512, 2], [132, 2], [1, D]])
                r_b = bass.AP(tensor=recip.tensor, offset=recip.offset,
                              ap=[recip.ap[0], [2, 2], [1, 2], [0, D]])
                nc.vector.tensor_mul(out=out_ap, in0=num_ap, in1=r_b)

                if OUT_SPLIT:
                    si = slab_idx[(i, kv_i, j)]
                    nc.sync.dma_start(
                        out=out_d[si],
                        in_=ost[:, j * G * CHUNK:(j + 1) * G * CHUNK])
                elif j == kk - 1:
                    si0 = slab_idx[(i, kv_i, 0)]
                    nc.sync.dma_start(
                        out=out_d[si0:si0 + kk].rearrange("k p c -> p k c"),
                        in_=ost[:].rearrange("p (k c) -> p k c", k=kk))

            pending = []
            for t in range(len(ikvs) + maxK - 1):
                cur = []
                for g in range(len(ikvs)):
                    j = t - g
                    i, kv_i = ikvs[g]
                    if 0 <= j < K[i]:
                        cur.append((i, kv_i, j, stage1(i, kv_i, j)))
                for (i, kv_i, j, pts) in pending:
                    stage2(i, kv_i, j, pts)
                pending = cur
            for (i, kv_i, j, pts) in pending:
                stage2(i, kv_i, j, pts)

    nc.finalize()
    return nc


# --------------------------------------------------------------------------
# entry point
# --------------------------------------------------------------------------

def kernel(query, key, value, decoder_segment_ids, _trace=False, _numpy=False,
           _qdt=QDT):
    query = np.asarray(query, np.float32)
    key = np.asarray(key, np.float32)
    value = np.asarray(value, np.float32)
    ids = np.asarray(decoder_segment_ids)
    # the block-diagonal decomposition relies on segment ids being sorted
    # (contiguous segments), as setup_inputs guarantees
    assert np.all(np.diff(ids.astype(np.int64), axis=-1) >= 0)

    runs, L, K, segs, slabs, chunks, nr, qbase, qcols = _structure(ids)
    core_ins = [_prepare_core(c, query, key, value, runs, L, K, segs, slabs,
                              chunks, nr, qbase, qcols, qdt=_qdt)
                for c in range(NCORES)]

    if _numpy:
        outs = [_numpy_schedule(ci, L, K, segs, slabs, chunks, nr, qbase)
                for ci in core_ins]
        return _assemble(outs, runs, slabs, nr)

    from concourse.bass_utils import run_bass_kernel_spmd

    cache_key = (tuple(L), _qdt)
    if cache_key not in _PROGRAM_CACHE:
        _PROGRAM_CACHE[cache_key] = _build_program(
            L, K, segs, slabs, chunks, nr, qbase, qcols, qdt=_qdt)
    nc = _PROGRAM_CACHE[cache_key]

    in_maps = [_pack_core(ci, K, slabs, chunks, nr, qbase, qdt=_qdt)
               for ci in core_ins]
    res = run_bass_kernel_spmd(nc, in_maps, list(range(NCORES)), trace=_trace)
    outs = [res.results[c]["out"] for c in range(NCORES)]
    full = _assemble(outs, runs, slabs, nr)
    if _trace:
        return full, res
    return full
